# revision 2
# baseline (speedup 1.0000x reference)
"""Trainium2 Bass kernel for nn_BandSplitDCTFilter.

Math: the reference's mirror-FFT DCT / band filter / inverse collapses to
    out_c = C1 (Z_c) C2^T - S1 (Z_c) S2^T,   Z_c = (A x_c A^T) .* W_eff_c
with A[k,j] = 2cos(pi k (2j+1)/128); C2/S2 carry the irfft half-spectrum
weights u_l and the 1/(4HW) scale; W_eff = pad(W_low)+pad(W_mid)+W_high
merges the three bands (they share the inverse basis under zero-padding).
Then y = x_out @ proj_w^T and LayerNorm.

Sharding: pure data-parallel, one sample per core (B=8 = 8 cores), small
weights replicated.

v2 (merged pipes): the two 128-channel half-pipes of v1 are merged into a
single c=256 pipeline.  The DRAM-mediated transposes (p1 between the two
forward DCTs, p2 between the two inverse transforms) now move 512-byte
contiguous runs instead of 256-byte ones, halving the descriptor count
that dominated the v1 profile (~24.5k -> ~12.3k entries; HBM descriptor
cost is latency-floor-bound below 512B).  Stages are chunk-pipelined:
p1 stores interleave with s2 drains per w0-quarter, s4/s5 run per
kh-half while the other half's pivot drains, p2 stores fire per s5
half, and s7/proj/LN/store proceed per j-quarter.  LN statistics are
taken on the proj PSUM tiles directly and the normalize fuses the
PSUM->SBUF drain (saves 32 ACT copies vs v1).
"""

import os

# The Bass kernel executes on the 8 axon-tunneled NeuronCores via PJRT;
# make sure jax can see them even if the caller pinned a platform default.
os.environ.setdefault("JAX_PLATFORMS", "axon,cpu")

import numpy as np
import ml_dtypes

import bass_rust
import concourse.bass as bass
import concourse.mybir as mybir
from concourse.tile import TileContext, ScopedClock
from concourse.bass_utils import run_bass_kernel_spmd

# ---------------------------------------------------------------------------
# Workarounds: this container's walrus rejects >1 sync wait per instruction.
# ---------------------------------------------------------------------------

_wait_ctr = 0


def _split_multi_waits(nc, max_waits=1):
    global _wait_ctr
    for f in nc.m.functions:
        for bb in f.blocks:
            out = []
            dirty = False
            for ins in bb.instructions:
                si = ins.sync_info
                if si is not None and len(si.on_wait) > max_waits:
                    waits = list(si.on_wait)
                    for w in waits[:-max_waits]:
                        _wait_ctr += 1
                        nop = bass_rust.InstNoOp(name=f"I-waitsplit-{_wait_ctr}")
                        nop.engine = ins.engine
                        nop.sync_info = mybir.SyncInfo(on_wait=[w], on_update=[])
                        out.append(nop)
                    ins.sync_info = mybir.SyncInfo(
                        on_wait=waits[-max_waits:], on_update=list(si.on_update)
                    )
                    dirty = True
                out.append(ins)
            if dirty:
                bb.instructions = out


def _patched_drain_and_barrier(self, tick_clock, wait_clock):
    nc = self.nc
    probe = nc.sync.nop(nofuse=True)
    wait_clock.add_sem_waits(probe.ins, ScopedClock({None: tick_clock.global_clock}))
    si = probe.ins.sync_info
    waits = list(si.on_wait) if si is not None else []
    probe.ins.sync_info = mybir.SyncInfo(on_wait=waits[:1], on_update=[])
    name2sem = {s.name: s for s in self.sems.allocated().values()}
    for w in waits[1:]:
        nc.sync.nop(nofuse=True)._wait_ge(name2sem[w.ant_name], w.wait_value)
    nc.sync.drain()
    nc.all_engine_barrier()
    popped = nc._tile_sem_poison_stack.pop()
    assert popped is self._sem_poison
    nc.clear_and_free_semaphores(list(self.sems.allocated().values()))
    nc.all_engine_barrier()


TileContext._drain_and_barrier = _patched_drain_and_barrier

# ---------------------------------------------------------------------------

B, H, W, C = 8, 64, 64, 256
N = H * W
F32 = mybir.dt.float32
BF16 = mybir.dt.bfloat16
ALU = mybir.AluOpType
ACTF = mybir.ActivationFunctionType


def _host_matrices():
    k = np.arange(64)
    j = np.arange(64)
    ang = np.pi * k[:, None] * (2 * j[None, :] + 1) / 128.0
    A = 2.0 * np.cos(ang)
    u = np.where(k == 0, 1.0, 2.0)
    C1T = np.cos(ang)
    S1T = np.sin(ang)
    C2T = u[:, None] * np.cos(ang) / 16384.0
    S2T = u[:, None] * np.sin(ang) / 16384.0

    AT = A.T.astype(np.float32)                                   # [h, k]
    khbd = np.zeros((128, 128), np.float32)
    khbd[0:64, 0:64] = AT
    khbd[64:128, 64:128] = AT
    cs2_half = np.concatenate([C2T, S2T], axis=1)                 # [kw, 128]
    cs2 = np.concatenate([cs2_half, cs2_half], axis=0)
    ICS = np.concatenate([C1T, -S1T], axis=0)
    return (khbd.astype(ml_dtypes.bfloat16),
            cs2.astype(ml_dtypes.bfloat16),
            np.ascontiguousarray(ICS.astype(ml_dtypes.bfloat16)))


_NC_CACHE = {}


def _build_nc(apply_gb):
    nc = bass.Bass(trn_type="TRN2")

    x_d = nc.dram_tensor("xr", [128, 8192], BF16, kind="ExternalInput")
    kh_d = nc.dram_tensor("kh", [128, 128], BF16, kind="ExternalInput")
    cs_d = nc.dram_tensor("cs", [128, 128], BF16, kind="ExternalInput")
    ics_d = nc.dram_tensor("ics", [128, 64], BF16, kind="ExternalInput")
    w_d = nc.dram_tensor("weff", [128, 8192], BF16, kind="ExternalInput")
    pjt_d = nc.dram_tensor("pjt", [128, 512], BF16, kind="ExternalInput")
    gb_d = nc.dram_tensor("gb", [2, 256], F32, kind="ExternalInput")
    y_d = nc.dram_tensor("y", [128, 8192], BF16, kind="ExternalOutput")

    with TileContext(nc) as tc:
        with (
            tc.tile_pool(name="consts", bufs=1) as consts,
            tc.tile_pool(name="wf", bufs=1) as wf,
            tc.tile_pool(name="xx", bufs=1) as xx,
            tc.tile_pool(name="t1", bufs=1) as t1p,
            tc.tile_pool(name="t2", bufs=1) as t2p_,
            tc.tile_pool(name="zp", bufs=1) as zpp,
            tc.tile_pool(name="u2", bufs=1) as u2p,
            tc.tile_pool(name="us", bufs=1) as usp,
            tc.tile_pool(name="x01", bufs=1) as x01p,
            tc.tile_pool(name="yr", bufs=1) as yr,
            tc.tile_pool(name="dramp", bufs=1, space="DRAM") as dramp,
            tc.tile_pool(name="ps", bufs=4, space="PSUM") as ps,
            tc.tile_pool(name="psy", bufs=4, space="PSUM") as psy,
            tc.tile_pool(name="small", bufs=8) as small,
        ):
            # ---- constants (gpsimd queue) ----
            khbd = consts.tile([128, 128], BF16, tag="khbd")
            cs2 = consts.tile([128, 128], BF16, tag="cs2")
            ics = consts.tile([128, 64], BF16, tag="ics")
            pjt = consts.tile([128, 512], BF16, tag="pjt")
            nc.gpsimd.dma_start(out=khbd[:], in_=kh_d[:])
            nc.gpsimd.dma_start(out=cs2[:], in_=cs_d[:])
            nc.gpsimd.dma_start(out=ics[:], in_=ics_d[:])
            nc.gpsimd.dma_start(out=pjt[:], in_=pjt_d[:])
            eps = consts.tile([128, 1], F32, tag="eps")
            nc.vector.memset(eps[:], 1e-5)
            weff = wf.tile([128, 8192], BF16, tag="wf")
            nc.gpsimd.dma_start(out=weff[:, 0:4096], in_=w_d[:, 0:4096])
            nc.gpsimd.dma_start(out=weff[:, 4096:8192], in_=w_d[:, 4096:8192])
            if apply_gb:
                gt = consts.tile([128, 256], F32, tag="gt")
                bt = consts.tile([128, 256], F32, tag="bt")
                gb_ap = gb_d.ap()
                g_b = bass.AP(tensor=gb_ap.tensor, offset=0, ap=[[0, 128], [1, 256]])
                b_b = bass.AP(tensor=gb_ap.tensor, offset=256, ap=[[0, 128], [1, 256]])
                nc.gpsimd.dma_start(out=gt[:], in_=g_b)
                nc.gpsimd.dma_start(out=bt[:], in_=b_b)

            # ---- tiles ----
            X = xx.tile([128, 8192], BF16, tag="xx")
            T1 = t1p.tile([128, 8192], BF16, tag="t1")
            T2p = t2p_.tile([128, 8192], BF16, tag="t2")
            Zp = zpp.tile([128, 8192], BF16, tag="zp")
            U2s = u2p.tile([128, 16384], BF16, tag="u2")
            Ustk = usp.tile([128, 16384], BF16, tag="us")
            X01A = x01p.tile([128, 4096], BF16, tag="x01a")
            X01B = x01p.tile([128, 4096], BF16, tag="x01b")
            Yraw = yr.tile([128, 8192], BF16, tag="yraw")
            D1 = dramp.tile([64, 16384], BF16, tag="d1")
            D2a = dramp.tile([64, 16384], BF16, tag="d2a")
            D2b = dramp.tile([64, 16384], BF16, tag="d2b")
            D2 = [D2a, D2b]

            # ---- x loads: 4 quarters (sync then scalar) ----
            for q in range(4):
                eng = nc.sync if q < 2 else nc.scalar
                eng.dma_start(out=X[:, q * 2048:(q + 1) * 2048],
                              in_=x_d[:, q * 2048:(q + 1) * 2048])

            # ---- s2: DCT-H, T1[(w1,kh),(w0,c)] = khbd^T @ X; p1 stores
            #      interleave per w0-quarter ----
            D1v = D1[:].rearrange("w (k c) -> k w c", c=256)
            for j in range(16):
                sl = slice(j * 512, (j + 1) * 512)
                pt = ps.tile([128, 512], F32, tag="ps")
                nc.tensor.matmul(pt[:], khbd[:], X[:, sl], start=True, stop=True)
                eng = nc.vector.tensor_copy if j % 4 == 0 else nc.scalar.copy
                eng(T1[:, sl], pt[:])
                if j % 4 == 3:
                    q = j // 4
                    for ks in range(2):
                        for w1 in range(2):
                            src = T1[w1 * 64 + ks * 32: w1 * 64 + ks * 32 + 32,
                                     q * 2048:(q + 1) * 2048]
                            dst = D1v[ks * 32: ks * 32 + 32,
                                      w1 * 32 + 8 * q: w1 * 32 + 8 * q + 8, :]
                            nc.sync.dma_start(
                                out=dst,
                                in_=src.rearrange("k (w c) -> k w c", c=256))

            # ---- p1 loads: T2p[(ks,w),(khh,c)], by (half, ks) ----
            for h in range(2):
                for ks in range(2):
                    dst = T2p[ks * 64:(ks + 1) * 64,
                              h * 4096:(h + 1) * 4096]
                    src = D1[:, ks * 8192 + h * 4096: ks * 8192 + (h + 1) * 4096]
                    nc.scalar.dma_start(out=dst, in_=src)

            # ---- s4: DCT-W + weff multiply -> Zp[(ks,kw),(khh,c)] ----
            for j in range(16):
                sl = slice(j * 512, (j + 1) * 512)
                pt = ps.tile([128, 512], F32, tag="ps")
                nc.tensor.matmul(pt[:], khbd[:], T2p[:, sl], start=True, stop=True)
                nc.vector.tensor_mul(Zp[:, sl], pt[:], weff[:, sl])

            # ---- s5: inverse-W (cs2), per ks-half; p2 stores per ks ----
            for ks in range(2):
                for jb in range(16):
                    sl = slice(jb * 512, (jb + 1) * 512)
                    pt = ps.tile([128, 512], F32, tag="ps")
                    nc.tensor.matmul(pt[:], cs2[ks * 64:(ks + 1) * 64, :],
                                     Zp[ks * 64:(ks + 1) * 64, sl],
                                     start=True, stop=True)
                    dsl = slice(ks * 8192 + jb * 512, ks * 8192 + (jb + 1) * 512)
                    eng = nc.vector.tensor_copy if jb % 4 == 0 else nc.scalar.copy
                    eng(U2s[:, dsl], pt[:])
                D2v = D2[ks]
                for jq in range(4):
                    for cshalf in range(2):
                        src = U2s[cshalf * 64 + jq * 16: cshalf * 64 + jq * 16 + 16,
                                  ks * 8192:(ks + 1) * 8192]
                        dst = D2v[cshalf * 32: cshalf * 32 + 32, :].rearrange(
                            "k (j c) -> j k c", c=256)[jq * 16: jq * 16 + 16]
                        nc.sync.dma_start(
                            out=dst,
                            in_=src.rearrange("j (k c) -> j k c", c=256))

            # ---- per j-quarter: p2 loads, s7, proj, LN, store ----
            mvall = small.tile([128, 64], F32, tag="mvall")
            rstdall = small.tile([128, 32], F32, tag="rstdall")
            nmrall = small.tile([128, 32], F32, tag="nmrall")
            mvv = mvall[:].rearrange("p (t x) -> p t x", x=2)

            for jq in range(4):
                for ks in range(2):
                    for cshalf in range(2):
                        dst = Ustk[cshalf * 64 + ks * 32: cshalf * 64 + ks * 32 + 32,
                                   jq * 4096:(jq + 1) * 4096]
                        src = D2[ks][cshalf * 32: cshalf * 32 + 32,
                                     jq * 4096:(jq + 1) * 4096]
                        nc.scalar.dma_start(out=dst, in_=src)

                # s7 for this quarter: t in [jq*16, jq*16+16), 2 groups of 8
                for g2 in range(2):
                    g = jq * 2 + g2
                    ptA = ps.tile([128, 512], F32, tag="ps", name=f"s7a{g}")
                    ptB = ps.tile([128, 512], F32, tag="ps", name=f"s7b{g}")
                    for nn in range(8):
                        t = g * 8 + nn
                        nc.tensor.matmul(ptA[:, nn * 64:(nn + 1) * 64],
                                         Ustk[:, t * 256: t * 256 + 128],
                                         ics[:], start=True, stop=True)
                        nc.tensor.matmul(ptB[:, nn * 64:(nn + 1) * 64],
                                         Ustk[:, t * 256 + 128:(t + 1) * 256],
                                         ics[:], start=True, stop=True)
                    eng = nc.vector.tensor_copy if g2 == 0 else nc.scalar.copy
                    eng(X01A[:, g * 512:(g + 1) * 512], ptA[:])
                    eng = nc.scalar.copy if g2 == 0 else nc.vector.tensor_copy
                    eng(X01B[:, g * 512:(g + 1) * 512], ptB[:])

                # proj + LN for t2 in [jq*8, jq*8+8)
                ptys = []
                for tp in range(4):
                    pty = psy.tile([128, 512], F32, tag="psy", name=f"py{jq}_{tp}")
                    ptys.append(pty)
                    for hh in range(2):
                        t2 = jq * 8 + tp * 2 + hh
                        osl = pty[:, hh * 256:(hh + 1) * 256]
                        nc.tensor.matmul(osl, X01A[:, t2 * 128:(t2 + 1) * 128],
                                         pjt[:, 0:256], start=True, stop=False)
                        nc.tensor.matmul(osl, X01B[:, t2 * 128:(t2 + 1) * 128],
                                         pjt[:, 256:512], start=False, stop=True)
                        stats = small.tile([128, 6], F32, tag="stats")
                        nc.vector.bn_stats(out=stats[:], in_=osl)
                        nc.vector.bn_aggr(out=mvall[:, t2 * 2:(t2 + 1) * 2],
                                          in_=stats[:])
                gs = slice(jq * 8, jq * 8 + 8)
                # std = sqrt(var + eps); rstd = 1/std; nmr = -mu*rstd
                nc.scalar.activation(out=rstdall[:, gs], in_=mvv[:, gs, 1],
                                     func=ACTF.Sqrt, bias=eps[:], scale=1.0)
                nc.vector.reciprocal(rstdall[:, gs], rstdall[:, gs])
                nc.vector.tensor_tensor(out=nmrall[:, gs], in0=mvv[:, gs, 0],
                                        in1=rstdall[:, gs], op=ALU.mult)
                nc.vector.tensor_scalar_mul(nmrall[:, gs], nmrall[:, gs], -1.0)
                for tp in range(4):
                    for hh in range(2):
                        t2 = jq * 8 + tp * 2 + hh
                        ysl = slice(t2 * 256, (t2 + 1) * 256)
                        nc.vector.tensor_scalar(
                            out=Yraw[:, ysl],
                            in0=ptys[tp][:, hh * 256:(hh + 1) * 256],
                            scalar1=rstdall[:, t2:t2 + 1],
                            scalar2=nmrall[:, t2:t2 + 1],
                            op0=ALU.mult, op1=ALU.add,
                        )
                        if apply_gb:
                            nc.vector.tensor_mul(Yraw[:, ysl], Yraw[:, ysl], gt[:])
                            nc.gpsimd.tensor_add(Yraw[:, ysl], Yraw[:, ysl], bt[:])
                nc.sync.dma_start(out=y_d[:, jq * 2048:(jq + 1) * 2048],
                                  in_=Yraw[:, jq * 2048:(jq + 1) * 2048])

    _split_multi_waits(nc)
    return nc


def _get_nc(apply_gb):
    key = bool(apply_gb)
    if key not in _NC_CACHE:
        _NC_CACHE[key] = _build_nc(key)
    return _NC_CACHE[key]


def _make_inputs(x, W_low, W_mid, W_high, proj_w, ln_g, ln_b):
    khbd, cs2, ICS = _host_matrices()

    W_eff = W_high[0].copy()
    W_eff[:32, :32] += W_mid[0]
    W_eff[:16, :16] += W_low[0]
    # weff layout: [(ks, kw), (khh, c)]
    weff = np.ascontiguousarray(
        W_eff.reshape(2, 32, 64, 256).transpose(0, 2, 1, 3)
        .reshape(128, 8192).astype(ml_dtypes.bfloat16))

    pjt = np.zeros((128, 512), ml_dtypes.bfloat16)
    pjt[:, :256] = proj_w.T[:128]
    pjt[:, 256:] = proj_w.T[128:]

    gb = np.stack([ln_g, ln_b]).astype(np.float32)
    consts = {"kh": khbd, "cs": cs2, "ics": ICS,
              "weff": weff, "pjt": pjt, "gb": gb}

    in_maps = []
    for b in range(B):
        m = dict(consts)
        # x layout: [(w1, h), (w0, c)]
        xp = x[b].reshape(64, 2, 32, 256).transpose(1, 0, 2, 3)
        m["xr"] = np.ascontiguousarray(
            xp.reshape(128, 8192).astype(ml_dtypes.bfloat16))
        in_maps.append(m)
    return in_maps


def kernel(x, W_low, W_mid, W_high, proj_w, ln_g, ln_b):
    x = np.ascontiguousarray(np.asarray(x, dtype=np.float32))
    W_low = np.asarray(W_low, dtype=np.float32)
    W_mid = np.asarray(W_mid, dtype=np.float32)
    W_high = np.asarray(W_high, dtype=np.float32)
    proj_w = np.asarray(proj_w, dtype=np.float32)
    ln_g = np.asarray(ln_g, dtype=np.float32)
    ln_b = np.asarray(ln_b, dtype=np.float32)

    apply_gb = not (np.all(ln_g == 1.0) and np.all(ln_b == 0.0))
    in_maps = _make_inputs(x, W_low, W_mid, W_high, proj_w, ln_g, ln_b)
    nc = _get_nc(apply_gb)
    res = run_bass_kernel_spmd(nc, in_maps, core_ids=list(range(B)))

    out = np.empty((B, N, C), np.float32)
    for b in range(B):
        yc = np.asarray(res.results[b]["y"]).astype(np.float32)
        yc = yc.reshape(128, 32, 256).transpose(1, 0, 2).reshape(4096, 256)
        out[b] = yc.reshape(64, 64, 256).transpose(1, 0, 2).reshape(4096, 256)
    return out


# revision 5
# speedup vs baseline: 1.0631x; 1.0631x over previous
"""Trainium2 Bass kernel for nn_BandSplitDCTFilter.

Math: the reference's mirror-FFT DCT / band filter / inverse collapses to
    out_c = C1 (Z_c) C2^T - S1 (Z_c) S2^T,   Z_c = (A x_c A^T) .* W_eff_c
with A[k,j] = 2cos(pi k (2j+1)/128); C2/S2 carry the irfft half-spectrum
weights u_l and the 1/(4HW) scale; W_eff = pad(W_low)+pad(W_mid)+W_high
merges the three bands (they share the inverse basis under zero-padding).
Then y = x_out @ proj_w^T and LayerNorm.

Sharding: pure data-parallel, one sample per core (B=8 = 8 cores), small
weights replicated.

v3: single merged c=256 pipeline (512B pivot runs, half the descriptor
count of the two-pipe v1), with every pipeline chunk in its OWN tile --
the Tile framework tracks dependencies at whole-tile granularity, so
chunk tiles are what make stores/loads/compute overlap:
  x quarters -> T1q[4] -> D1k[2] -> T2h[2] -> Zh[2] -> U2k[2] ->
  D2kc[2][2] -> Usq[4] -> X01[A/B]q[4] -> Yq[4]
LN is three-way split: even t2 tiles use vector bn_stats + a fused
PSUM->SBUF normalize (tensor_scalar); odd t2 tiles compute sum/sumsq on
the scalar engine via activation accum_out (the Copy pass doubles as the
PSUM drain) and normalize in-place on gpsimd, which otherwise idles.
PSUM->SBUF drains alternate vector/scalar.
"""

import os

os.environ.setdefault("JAX_PLATFORMS", "axon,cpu")

import numpy as np
import ml_dtypes

import bass_rust
import concourse.bass as bass
import concourse.mybir as mybir
from concourse.tile import TileContext, ScopedClock
from concourse.bass_utils import run_bass_kernel_spmd

# ---------------------------------------------------------------------------
# Workarounds: this container's walrus rejects >1 sync wait per instruction.
# ---------------------------------------------------------------------------

_wait_ctr = 0


def _split_multi_waits(nc, max_waits=1):
    global _wait_ctr
    for f in nc.m.functions:
        for bb in f.blocks:
            out = []
            dirty = False
            for ins in bb.instructions:
                si = ins.sync_info
                if si is not None and len(si.on_wait) > max_waits:
                    waits = list(si.on_wait)
                    for w in waits[:-max_waits]:
                        _wait_ctr += 1
                        nop = bass_rust.InstNoOp(name=f"I-waitsplit-{_wait_ctr}")
                        nop.engine = ins.engine
                        nop.sync_info = mybir.SyncInfo(on_wait=[w], on_update=[])
                        out.append(nop)
                    ins.sync_info = mybir.SyncInfo(
                        on_wait=waits[-max_waits:], on_update=list(si.on_update)
                    )
                    dirty = True
                out.append(ins)
            if dirty:
                bb.instructions = out


def _patched_drain_and_barrier(self, tick_clock, wait_clock):
    nc = self.nc
    probe = nc.sync.nop(nofuse=True)
    wait_clock.add_sem_waits(probe.ins, ScopedClock({None: tick_clock.global_clock}))
    si = probe.ins.sync_info
    waits = list(si.on_wait) if si is not None else []
    probe.ins.sync_info = mybir.SyncInfo(on_wait=waits[:1], on_update=[])
    name2sem = {s.name: s for s in self.sems.allocated().values()}
    for w in waits[1:]:
        nc.sync.nop(nofuse=True)._wait_ge(name2sem[w.ant_name], w.wait_value)
    nc.sync.drain()
    nc.all_engine_barrier()
    popped = nc._tile_sem_poison_stack.pop()
    assert popped is self._sem_poison
    nc.clear_and_free_semaphores(list(self.sems.allocated().values()))
    nc.all_engine_barrier()


TileContext._drain_and_barrier = _patched_drain_and_barrier

# ---------------------------------------------------------------------------

B, H, W, C = 8, 64, 64, 256
N = H * W
F32 = mybir.dt.float32
BF16 = mybir.dt.bfloat16
ALU = mybir.AluOpType
ACTF = mybir.ActivationFunctionType


def _host_matrices():
    k = np.arange(64)
    j = np.arange(64)
    ang = np.pi * k[:, None] * (2 * j[None, :] + 1) / 128.0
    A = 2.0 * np.cos(ang)
    u = np.where(k == 0, 1.0, 2.0)
    C1T = np.cos(ang)
    S1T = np.sin(ang)
    C2T = u[:, None] * np.cos(ang) / 16384.0
    S2T = u[:, None] * np.sin(ang) / 16384.0

    AT = A.T.astype(np.float32)                                   # [h, k]
    khbd = np.zeros((128, 128), np.float32)
    khbd[0:64, 0:64] = AT
    khbd[64:128, 64:128] = AT
    cs2_half = np.concatenate([C2T, S2T], axis=1)                 # [kw, 128]
    cs2 = np.concatenate([cs2_half, cs2_half], axis=0)
    ICS = np.concatenate([C1T, -S1T], axis=0)
    return (khbd.astype(ml_dtypes.bfloat16),
            cs2.astype(ml_dtypes.bfloat16),
            np.ascontiguousarray(ICS.astype(ml_dtypes.bfloat16)))


_NC_CACHE = {}


def _build_nc(apply_gb):
    nc = bass.Bass(trn_type="TRN2")

    x_d = nc.dram_tensor("xr", [128, 8192], BF16, kind="ExternalInput")
    kh_d = nc.dram_tensor("kh", [128, 128], BF16, kind="ExternalInput")
    cs_d = nc.dram_tensor("cs", [128, 128], BF16, kind="ExternalInput")
    ics_d = nc.dram_tensor("ics", [128, 64], BF16, kind="ExternalInput")
    w_d = nc.dram_tensor("weff", [128, 8192], BF16, kind="ExternalInput")
    pjt_d = nc.dram_tensor("pjt", [128, 512], BF16, kind="ExternalInput")
    gb_d = nc.dram_tensor("gb", [2, 256], F32, kind="ExternalInput")
    y_d = nc.dram_tensor("y", [128, 8192], BF16, kind="ExternalOutput")

    with TileContext(nc) as tc:
        with (
            tc.tile_pool(name="consts", bufs=1) as consts,
            tc.tile_pool(name="wf", bufs=1) as wf,
            tc.tile_pool(name="xx", bufs=1) as xx,
            tc.tile_pool(name="t1", bufs=1) as t1p,
            tc.tile_pool(name="t2", bufs=1) as t2p_,
            tc.tile_pool(name="zp", bufs=1) as zpp,
            tc.tile_pool(name="u2", bufs=1) as u2p,
            tc.tile_pool(name="us", bufs=1) as usp,
            tc.tile_pool(name="x01", bufs=1) as x01p,
            tc.tile_pool(name="yr", bufs=1) as yr,
            tc.tile_pool(name="dramp", bufs=1, space="DRAM") as dramp,
            tc.tile_pool(name="ps", bufs=4, space="PSUM") as ps,
            tc.tile_pool(name="psy", bufs=4, space="PSUM") as psy,
            tc.tile_pool(name="small", bufs=16) as small,
        ):
            # ---- constants (gpsimd queue) ----
            khbd = consts.tile([128, 128], BF16, tag="khbd")
            cs2 = consts.tile([128, 128], BF16, tag="cs2")
            ics = consts.tile([128, 64], BF16, tag="ics")
            pjt = consts.tile([128, 512], BF16, tag="pjt")
            nc.gpsimd.dma_start(out=khbd[:], in_=kh_d[:])
            nc.gpsimd.dma_start(out=cs2[:], in_=cs_d[:])
            nc.gpsimd.dma_start(out=ics[:], in_=ics_d[:])
            nc.gpsimd.dma_start(out=pjt[:], in_=pjt_d[:])
            eps = consts.tile([128, 1], F32, tag="eps")
            nc.vector.memset(eps[:], 1e-5)
            i256 = consts.tile([128, 1], F32, tag="i256")
            nc.vector.memset(i256[:], 1.0 / 256.0)
            weff = wf.tile([128, 8192], BF16, tag="wf")
            nc.gpsimd.dma_start(out=weff[:, 0:4096], in_=w_d[:, 0:4096])
            nc.gpsimd.dma_start(out=weff[:, 4096:8192], in_=w_d[:, 4096:8192])
            if apply_gb:
                gt = consts.tile([128, 256], F32, tag="gt")
                bt = consts.tile([128, 256], F32, tag="bt")
                gb_ap = gb_d.ap()
                g_b = bass.AP(tensor=gb_ap.tensor, offset=0, ap=[[0, 128], [1, 256]])
                b_b = bass.AP(tensor=gb_ap.tensor, offset=256, ap=[[0, 128], [1, 256]])
                nc.gpsimd.dma_start(out=gt[:], in_=g_b)
                nc.gpsimd.dma_start(out=bt[:], in_=b_b)

            # ---- per-chunk tiles ----
            X = xx.tile([128, 8192], BF16, tag="xx")
            T1q = [t1p.tile([128, 2048], BF16, tag=f"t1_{q}", name=f"T1q{q}")
                   for q in range(4)]
            T2h = [t2p_.tile([128, 4096], BF16, tag=f"t2_{h}", name=f"T2h{h}")
                   for h in range(2)]
            Zh = [zpp.tile([128, 4096], BF16, tag=f"zp_{h}", name=f"Zh{h}")
                  for h in range(2)]
            U2k = [u2p.tile([128, 8192], BF16, tag=f"u2_{k}", name=f"U2k{k}")
                   for k in range(2)]
            Usq = [usp.tile([128, 4096], BF16, tag=f"us_{q}", name=f"Usq{q}")
                   for q in range(4)]
            X01A = [x01p.tile([128, 1024], BF16, tag=f"xa_{q}", name=f"X01A{q}")
                    for q in range(4)]
            X01B = [x01p.tile([128, 1024], BF16, tag=f"xb_{q}", name=f"X01B{q}")
                    for q in range(4)]
            Yq = [yr.tile([128, 2048], BF16, tag=f"yq_{q}", name=f"Yq{q}")
                  for q in range(4)]
            D1k = [dramp.tile([64, 8192], BF16, tag=f"d1_{k}", name=f"D1k{k}")
                   for k in range(2)]
            D2kc = [[dramp.tile([32, 16384], BF16, tag=f"d2_{k}{c}",
                                name=f"D2k{k}c{c}") for c in range(2)]
                    for k in range(2)]

            # ---- x loads: 4 quarters ----
            for q in range(4):
                eng = nc.sync if q < 2 else nc.scalar
                eng.dma_start(out=X[:, q * 2048:(q + 1) * 2048],
                              in_=x_d[:, q * 2048:(q + 1) * 2048])

            # ---- s2 (DCT-H) + p1 stores per w0-quarter ----
            for j in range(16):
                sl = slice(j * 512, (j + 1) * 512)
                q, jj = j // 4, j % 4
                pt = ps.tile([128, 512], F32, tag="ps")
                nc.tensor.matmul(pt[:], khbd[:], X[:, sl], start=True, stop=True)
                eng = nc.vector.tensor_copy if j % 2 == 0 else nc.scalar.copy
                eng(T1q[q][:, jj * 512:(jj + 1) * 512], pt[:])
                if jj == 3:
                    for ks in range(2):
                        D1kv = D1k[ks][:].rearrange("w (k c) -> k w c", c=256)
                        for w1 in range(2):
                            src = T1q[q][w1 * 64 + ks * 32: w1 * 64 + ks * 32 + 32, :]
                            dst = D1kv[:, w1 * 32 + 8 * q: w1 * 32 + 8 * q + 8, :]
                            nc.sync.dma_start(
                                out=dst,
                                in_=src.rearrange("k (w c) -> k w c", c=256))

            # ---- p1 loads: T2h[h][(ks,w), (khh_h, c)] ----
            for h in range(2):
                for ks in range(2):
                    nc.scalar.dma_start(
                        out=T2h[h][ks * 64:(ks + 1) * 64, :],
                        in_=D1k[ks][:, h * 4096:(h + 1) * 4096])

            # ---- s4 (DCT-W) + weff multiply -> Zh ----
            for j in range(16):
                h, jj = j // 8, j % 8
                pt = ps.tile([128, 512], F32, tag="ps")
                nc.tensor.matmul(pt[:], khbd[:],
                                 T2h[h][:, jj * 512:(jj + 1) * 512],
                                 start=True, stop=True)
                nc.vector.tensor_mul(Zh[h][:, jj * 512:(jj + 1) * 512], pt[:],
                                     weff[:, j * 512:(j + 1) * 512])

            # ---- s5 (inverse-W) per ks-half; p2 stores per ks ----
            for ks in range(2):
                for jb in range(16):
                    h, bb = jb // 8, jb % 8
                    pt = ps.tile([128, 512], F32, tag="ps")
                    nc.tensor.matmul(pt[:], cs2[ks * 64:(ks + 1) * 64, :],
                                     Zh[h][ks * 64:(ks + 1) * 64,
                                           bb * 512:(bb + 1) * 512],
                                     start=True, stop=True)
                    dsl = slice(jb * 512, (jb + 1) * 512)
                    eng = nc.vector.tensor_copy if jb % 2 == 0 else nc.scalar.copy
                    eng(U2k[ks][:, dsl], pt[:])
                for jq in range(4):
                    for cshalf in range(2):
                        src = U2k[ks][cshalf * 64 + jq * 16:
                                      cshalf * 64 + jq * 16 + 16, :]
                        dst = D2kc[ks][cshalf][:].rearrange(
                            "k (j c) -> j k c", c=256)[jq * 16: jq * 16 + 16]
                        nc.sync.dma_start(
                            out=dst,
                            in_=src.rearrange("j (k c) -> j k c", c=256))

            # ---- per j-quarter: p2 loads, s7, proj, LN, store ----
            for jq in range(4):
                for ks in range(2):
                    for cshalf in range(2):
                        nc.scalar.dma_start(
                            out=Usq[jq][cshalf * 64 + ks * 32:
                                        cshalf * 64 + ks * 32 + 32, :],
                            in_=D2kc[ks][cshalf][:, jq * 4096:(jq + 1) * 4096])

                # s7: t in [jq*16, jq*16+16), 2 groups of 8
                for g2 in range(2):
                    ptA = ps.tile([128, 512], F32, tag="ps", name=f"s7a{jq}{g2}")
                    ptB = ps.tile([128, 512], F32, tag="ps", name=f"s7b{jq}{g2}")
                    for nn in range(8):
                        tl = (g2 * 8 + nn) * 256
                        nc.tensor.matmul(ptA[:, nn * 64:(nn + 1) * 64],
                                         Usq[jq][:, tl: tl + 128],
                                         ics[:], start=True, stop=True)
                        nc.tensor.matmul(ptB[:, nn * 64:(nn + 1) * 64],
                                         Usq[jq][:, tl + 128: tl + 256],
                                         ics[:], start=True, stop=True)
                    eng = nc.vector.tensor_copy if g2 == 0 else nc.scalar.copy
                    eng(X01A[jq][:, g2 * 512:(g2 + 1) * 512], ptA[:])
                    eng = nc.scalar.copy if g2 == 0 else nc.vector.tensor_copy
                    eng(X01B[jq][:, g2 * 512:(g2 + 1) * 512], ptB[:])

                # proj + LN for tt in [0, 8)  (t2 = jq*8 + tt)
                mvq = small.tile([128, 16], F32, tag=f"mv{jq}", name=f"mv{jq}")
                s0q = small.tile([128, 8], F32, tag=f"s0{jq}", name=f"s0{jq}")
                s1q = small.tile([128, 8], F32, tag=f"s1{jq}", name=f"s1{jq}")
                rstdq = small.tile([128, 8], F32, tag=f"rs{jq}", name=f"rs{jq}")
                nmrq = small.tile([128, 8], F32, tag=f"nm{jq}", name=f"nm{jq}")
                mvv = mvq[:].rearrange("p (t x) -> p t x", x=2)
                ptys = []
                for tp in range(4):
                    pty = psy.tile([128, 512], F32, tag="psy", name=f"py{jq}{tp}")
                    ptys.append(pty)
                    for hh in range(2):
                        tt = tp * 2 + hh
                        t2 = jq * 8 + tt
                        osl = pty[:, hh * 256:(hh + 1) * 256]
                        nc.tensor.matmul(osl, X01A[jq][:, tt * 128:(tt + 1) * 128],
                                         pjt[:, 0:256], start=True, stop=False)
                        nc.tensor.matmul(osl, X01B[jq][:, tt * 128:(tt + 1) * 128],
                                         pjt[:, 256:512], start=False, stop=True)
                        if tt % 2 == 0:
                            # vector path: bn stats (packed at col tt//2)
                            vi = tt // 2
                            stats = small.tile([128, 6], F32, tag="stats")
                            nc.vector.bn_stats(out=stats[:], in_=osl)
                            nc.vector.bn_aggr(out=mvq[:, vi * 2: vi * 2 + 2],
                                              in_=stats[:])
                        else:
                            # scalar path: raw drain + sum, then sumsq
                            oi = tt // 2
                            ysl = slice(tt * 256, (tt + 1) * 256)
                            nc.scalar.activation(
                                out=Yq[jq][:, ysl], in_=osl, func=ACTF.Copy,
                                accum_out=s0q[:, oi: oi + 1])
                            sq = small.tile([128, 256], BF16, tag="sqscr")
                            nc.scalar.activation(
                                out=sq[:], in_=osl, func=ACTF.Square,
                                accum_out=s1q[:, oi: oi + 1])
                # vector path: rstd = 1/sqrt(var+eps), nmr = -mu*rstd
                nc.scalar.activation(out=rstdq[:, 0:4],
                                     in_=mvv[:, 0:4, 1], func=ACTF.Sqrt,
                                     bias=eps[:], scale=1.0)
                nc.vector.reciprocal(rstdq[:, 0:4], rstdq[:, 0:4])
                nc.vector.tensor_tensor(out=nmrq[:, 0:4], in0=mvv[:, 0:4, 0],
                                        in1=rstdq[:, 0:4], op=ALU.mult)
                nc.vector.tensor_scalar_mul(nmrq[:, 0:4], nmrq[:, 0:4], -1.0)
                # scalar path: mu = s0/256, var = s1/256 - mu^2
                mu_t = s0q[:, 4:8]
                ey_t = s1q[:, 4:8]
                nc.vector.tensor_scalar_mul(mu_t, s0q[:, 0:4], i256[:])
                nc.vector.tensor_scalar_mul(ey_t, s1q[:, 0:4], i256[:])
                nc.vector.tensor_tensor(out=s0q[:, 0:4], in0=mu_t, in1=mu_t,
                                        op=ALU.mult)
                nc.vector.tensor_tensor(out=ey_t, in0=ey_t, in1=s0q[:, 0:4],
                                        op=ALU.subtract)
                nc.scalar.activation(out=rstdq[:, 4:8], in_=ey_t,
                                     func=ACTF.Sqrt, bias=eps[:], scale=1.0)
                nc.vector.reciprocal(rstdq[:, 4:8], rstdq[:, 4:8])
                nc.vector.tensor_tensor(out=nmrq[:, 4:8], in0=mu_t,
                                        in1=rstdq[:, 4:8], op=ALU.mult)
                nc.vector.tensor_scalar_mul(nmrq[:, 4:8], nmrq[:, 4:8], -1.0)

                for tp in range(4):
                    for hh in range(2):
                        tt = tp * 2 + hh
                        ysl = slice(tt * 256, (tt + 1) * 256)
                        if tt % 2 == 0:
                            ci = tt // 2
                            nc.vector.tensor_scalar(
                                out=Yq[jq][:, ysl],
                                in0=ptys[tp][:, hh * 256:(hh + 1) * 256],
                                scalar1=rstdq[:, ci: ci + 1],
                                scalar2=nmrq[:, ci: ci + 1],
                                op0=ALU.mult, op1=ALU.add,
                            )
                        else:
                            ci = 4 + tt // 2
                            nc.gpsimd.tensor_scalar(
                                out=Yq[jq][:, ysl], in0=Yq[jq][:, ysl],
                                scalar1=rstdq[:, ci: ci + 1],
                                scalar2=nmrq[:, ci: ci + 1],
                                op0=ALU.mult, op1=ALU.add,
                            )
                        if apply_gb:
                            nc.vector.tensor_mul(Yq[jq][:, ysl],
                                                 Yq[jq][:, ysl], gt[:])
                            nc.gpsimd.tensor_add(Yq[jq][:, ysl],
                                                 Yq[jq][:, ysl], bt[:])
                nc.sync.dma_start(out=y_d[:, jq * 2048:(jq + 1) * 2048],
                                  in_=Yq[jq][:])

    _split_multi_waits(nc)
    return nc


def _get_nc(apply_gb):
    key = bool(apply_gb)
    if key not in _NC_CACHE:
        _NC_CACHE[key] = _build_nc(key)
    return _NC_CACHE[key]


def _make_inputs(x, W_low, W_mid, W_high, proj_w, ln_g, ln_b):
    khbd, cs2, ICS = _host_matrices()

    W_eff = W_high[0].copy()
    W_eff[:32, :32] += W_mid[0]
    W_eff[:16, :16] += W_low[0]
    # weff layout: [(ks, kw), (khh, c)]
    weff = np.ascontiguousarray(
        W_eff.reshape(2, 32, 64, 256).transpose(0, 2, 1, 3)
        .reshape(128, 8192).astype(ml_dtypes.bfloat16))

    pjt = np.zeros((128, 512), ml_dtypes.bfloat16)
    pjt[:, :256] = proj_w.T[:128]
    pjt[:, 256:] = proj_w.T[128:]

    gb = np.stack([ln_g, ln_b]).astype(np.float32)
    consts = {"kh": khbd, "cs": cs2, "ics": ICS,
              "weff": weff, "pjt": pjt, "gb": gb}

    in_maps = []
    for b in range(B):
        m = dict(consts)
        # x layout: [(w1, h), (w0, c)]
        xp = x[b].reshape(64, 2, 32, 256).transpose(1, 0, 2, 3)
        m["xr"] = np.ascontiguousarray(
            xp.reshape(128, 8192).astype(ml_dtypes.bfloat16))
        in_maps.append(m)
    return in_maps


def kernel(x, W_low, W_mid, W_high, proj_w, ln_g, ln_b):
    x = np.ascontiguousarray(np.asarray(x, dtype=np.float32))
    W_low = np.asarray(W_low, dtype=np.float32)
    W_mid = np.asarray(W_mid, dtype=np.float32)
    W_high = np.asarray(W_high, dtype=np.float32)
    proj_w = np.asarray(proj_w, dtype=np.float32)
    ln_g = np.asarray(ln_g, dtype=np.float32)
    ln_b = np.asarray(ln_b, dtype=np.float32)

    apply_gb = not (np.all(ln_g == 1.0) and np.all(ln_b == 0.0))
    in_maps = _make_inputs(x, W_low, W_mid, W_high, proj_w, ln_g, ln_b)
    nc = _get_nc(apply_gb)
    res = run_bass_kernel_spmd(nc, in_maps, core_ids=list(range(B)))

    out = np.empty((B, N, C), np.float32)
    for b in range(B):
        yc = np.asarray(res.results[b]["y"]).astype(np.float32)
        yc = yc.reshape(128, 32, 256).transpose(1, 0, 2).reshape(4096, 256)
        out[b] = yc.reshape(64, 64, 256).transpose(1, 0, 2).reshape(4096, 256)
    return out
